# revision 4
# baseline (speedup 1.0000x reference)
"""Chamfer loss (adapted) on 8 TRN2 NeuronCores via Bass/Tile.

Problem: B=2, N=16384, M=8192, D=3
  w = softmax(weights, axis=1)
  dist[b,n,m] = ||p1[b,n] - p2[b,m]||^2  (via sq1 + sq2 - 2*cross)
  loss = mean_b( sum_n w*min_m dist + mean_m min_n dist )

Sharding: core c -> batch b = c//4, quarter q = c%4.
  mm1: rows = p1[b, q*4096:(q+1)*4096] vs all M points2 -> min over free dim
  mm2: rows = p2[b, q*2048:(q+1)*2048] vs all N points1 -> min over free dim
No collectives: each core emits a partial scalar; host sums 8 partials / B.

Numerics: distances need ~1e-4 abs accuracy but are computed via terms of
magnitude ~O(10) (catastrophic cancellation), so the cross term cannot use
raw bf16 matmul. Each coordinate x is split hi/lo (x ~= xh + xl, both bf16);
the K=15 augmented contraction computes
  R[n,m] = -2*sum_d x_d*y_d + ||y||^2   exactly over the bf16-split points,
accumulated in fp32 PSUM (bf16*bf16 products are exact in fp32). min1[n] =
sq1[n] + min_m R[n,m]. TensorE streams 1 column/cycle regardless of K, so
K=15 costs the same as K=5 but keeps full precision.

Reduction (the bottleneck): TensorE produces 512 fp32/instr but
tensor_reduce runs at 1 elem/cycle/lane on VectorE. Per 2048-col block
either:
  path A: DVE reduce_min straight from PSUM (fp32)           ~2258ns
  path B: ScalarE Identity(+sq1 bias) PSUM->SBUF bf16 convert ~1850ns
          + DVE bf16 2x-mode pairwise-min tournament          ~1120ns
Mixing A and B balances ScalarE vs VectorE so both engines stay busy.
"""

import os
import numpy as np
import ml_dtypes

bf16 = ml_dtypes.bfloat16

B, N, M, D = 2, 16384, 8192, 3
NSH, MSH = N // 4, M // 4          # 4096 query rows (mm1), 2048 (mm2) per core
K = 15                             # augmented contraction depth
BLK = 2048                         # free-dim columns per PSUM block
NRT1, NBLK1 = NSH // 128, M // BLK     # 32 row-tiles x 4 blocks  (mm1)
NRT2, NBLK2 = MSH // 128, N // BLK     # 16 row-tiles x 8 blocks  (mm2)

# Which block index inside each row-tile uses path A (DVE-direct reduce).
PATH_A_BLOCKS_MM1 = (0,)
PATH_A_BLOCKS_MM2 = (0,)

_compiled = None
_last_results = None


def _build():
    from contextlib import ExitStack
    import concourse.mybir as mybir
    import concourse.tile as tile
    from concourse import bacc
    from concourse.masks import make_identity

    f32, bf = mybir.dt.float32, mybir.dt.bfloat16
    X = mybir.AxisListType.X
    MIN, ADD, MULT = mybir.AluOpType.min, mybir.AluOpType.add, mybir.AluOpType.mult
    IDENT, EXP = mybir.ActivationFunctionType.Identity, mybir.ActivationFunctionType.Exp

    nc = bacc.Bacc("TRN2", target_bir_lowering=False, debug=False)

    q1 = nc.dram_tensor("q1", (K, NSH), bf, kind="ExternalInput").ap()
    r2 = nc.dram_tensor("r2", (K, M), bf, kind="ExternalInput").ap()
    q2 = nc.dram_tensor("q2", (K, MSH), bf, kind="ExternalInput").ap()
    r1 = nc.dram_tensor("r1", (K, N), bf, kind="ExternalInput").ap()
    s1a = nc.dram_tensor("s1a", (128, NRT1), f32, kind="ExternalInput").ap()
    s2a = nc.dram_tensor("s2a", (128, NRT2), f32, kind="ExternalInput").ap()
    wmat = nc.dram_tensor("wmat", (128, 128), f32, kind="ExternalInput").ap()
    wsh = nc.dram_tensor("wsh", (NSH // 128, 128), f32, kind="ExternalInput").ap()
    out = nc.dram_tensor("out", (1, 1), f32, kind="ExternalOutput").ap()

    with tile.TileContext(nc) as tc, ExitStack() as ctx:
        const = ctx.enter_context(tc.tile_pool(name="const", bufs=1))
        psum = ctx.enter_context(tc.tile_pool(name="psum", bufs=2, space="PSUM"))
        conv = ctx.enter_context(tc.tile_pool(name="conv", bufs=3))
        trn = ctx.enter_context(tc.tile_pool(name="trn", bufs=2))
        coll = ctx.enter_context(tc.tile_pool(name="coll", bufs=2))

        q1t = const.tile([K, NSH], bf, tag="q1t")
        nc.sync.dma_start(q1t[:], q1[:])
        r2t = const.tile([K, M], bf, tag="r2t")
        nc.sync.dma_start(r2t[:], r2[:])
        q2t = const.tile([K, MSH], bf, tag="q2t")
        nc.sync.dma_start(q2t[:], q2[:])
        r1t = const.tile([K, N], bf, tag="r1t")
        nc.sync.dma_start(r1t[:], r1[:])
        s1t = const.tile([128, NRT1], f32, tag="s1t")
        nc.sync.dma_start(s1t[:], s1a[:])
        s2t = const.tile([128, NRT2], f32, tag="s2t")
        nc.sync.dma_start(s2t[:], s2a[:])
        wmt = const.tile([128, 128], f32, tag="wmt")
        nc.sync.dma_start(wmt[:], wmat[:])
        wst = const.tile([NSH // 128, 128], f32, tag="wst")
        nc.sync.dma_start(wst[:], wsh[:])

        min1 = const.tile([128, NRT1], f32, tag="min1")
        min2 = const.tile([128, NRT2], f32, tag="min2")

        def do_matrix(qt, rhs, rt_cnt, blk_cnt, bias, out_min, a_blocks):
            for rt in range(rt_cnt):
                lhsT = qt[:, rt * 128:(rt + 1) * 128]
                bias_col = bias[:, rt:rt + 1]
                n_b = blk_cnt - len(a_blocks)
                collB = coll.tile([128, 8 * 256], bf, tag="collB")
                collA = coll.tile([128, 8], f32, tag="collA")
                ia = ib = 0
                for j in range(blk_cnt):
                    ps = psum.tile([128, BLK], f32, tag="blk")
                    for k in range(4):
                        nc.tensor.matmul(
                            ps[:, k * 512:(k + 1) * 512], lhsT,
                            rhs[:, (j * 4 + k) * 512:(j * 4 + k + 1) * 512],
                            start=True, stop=True)
                    if j in a_blocks:
                        nc.vector.tensor_reduce(
                            collA[:, ia:ia + 1], ps[:], axis=X, op=MIN)
                        ia += 1
                    else:
                        cv = conv.tile([128, BLK], bf, tag="cv")
                        nc.scalar.activation(cv[:], ps[:], IDENT,
                                             bias=bias_col, scale=1.0)
                        t1 = trn.tile([128, 1024], bf, tag="t1")
                        nc.vector.tensor_tensor(
                            t1[:], cv[:, 0:1024], cv[:, 1024:2048], op=MIN)
                        t2 = trn.tile([128, 512], bf, tag="t2")
                        nc.vector.tensor_tensor(
                            t2[:], t1[:, 0:512], t1[:, 512:1024], op=MIN)
                        nc.vector.tensor_tensor(
                            collB[:, ib * 256:(ib + 1) * 256],
                            t2[:, 0:256], t2[:, 256:512], op=MIN)
                        ib += 1
                rB = coll.tile([128, 1], f32, tag="rB")
                nc.vector.tensor_reduce(
                    rB[:], collB[:, 0:n_b * 256], axis=X, op=MIN)
                if ia == 1:
                    rA = collA[:, 0:1]
                elif ia > 1:
                    rA = coll.tile([128, 1], f32, tag="rA")
                    nc.vector.tensor_reduce(
                        rA[:], collA[:, 0:ia], axis=X, op=MIN)
                # out = min(rA + bias, rB)   (pathB mins already biased)
                nc.vector.scalar_tensor_tensor(
                    out_min[:, rt:rt + 1], rA, bias_col, rB[:],
                    op0=ADD, op1=MIN)

        do_matrix(q1t, r2t, NRT1, NBLK1, s1t, min1, PATH_A_BLOCKS_MM1)
        do_matrix(q2t, r1t, NRT2, NBLK2, s2t, min2, PATH_A_BLOCKS_MM2)

        # ---- finalization: softmax weights + weighted sums -> scalar ----
        ident = const.tile([128, 128], f32, tag="ident")
        make_identity(nc, ident[:])
        ewm = const.tile([128, 128], f32, tag="ewm")
        nc.scalar.activation(ewm[:], wmt[:], EXP)
        zcol = const.tile([128, 1], f32, tag="zcol")
        nc.vector.tensor_reduce(zcol[:], ewm[:], axis=X, op=ADD)
        ones = const.tile([128, 1], f32, tag="ones")
        nc.gpsimd.memset(ones[:], 1.0)

        wse = const.tile([NSH // 128, 128], f32, tag="wse")
        nc.scalar.activation(wse[:], wst[:], EXP)

        pz = psum.tile([128, BLK], f32, tag="blk")
        # Z = sum_n exp(w[n])    (cross-partition sum via PE)
        nc.tensor.matmul(pz[0:1, 0:1], zcol[:], ones[:], start=True, stop=True)
        # exp(w_shard) transposed into min1's [p, rt] layout
        nc.tensor.transpose(pz[0:128, 512:512 + NSH // 128], wse[:],
                            ident[0:NSH // 128, 0:NSH // 128])
        ewsh = const.tile([128, NRT1], f32, tag="ewsh")
        nc.scalar.copy(ewsh[:], pz[0:128, 512:512 + NRT1])

        tmp = const.tile([128, NRT1], f32, tag="tmp")
        t1v = const.tile([128, 1], f32, tag="t1v")
        nc.vector.scalar_tensor_tensor(
            tmp[:], ewsh[:], 1.0, min1[:], op0=MULT, op1=MULT,
            accum_out=t1v[:])
        t2v = const.tile([128, 1], f32, tag="t2v")
        nc.vector.tensor_reduce(t2v[:], min2[:], axis=X, op=ADD)
        nc.tensor.matmul(pz[0:1, 1024:1025], t1v[:], ones[:],
                         start=True, stop=True)
        nc.tensor.matmul(pz[0:1, 1536:1537], t2v[:], ones[:],
                         start=True, stop=True)

        fin = const.tile([1, 4], f32, tag="fin")
        nc.scalar.copy(fin[0:1, 0:1], pz[0:1, 0:1])
        nc.scalar.copy(fin[0:1, 1:2], pz[0:1, 1024:1025])
        nc.scalar.copy(fin[0:1, 2:3], pz[0:1, 1536:1537])
        zr = const.tile([1, 1], f32, tag="zr")
        nc.vector.reciprocal(zr[:], fin[0:1, 0:1])
        p1t = const.tile([1, 1], f32, tag="p1t")
        nc.vector.tensor_mul(p1t[:], fin[0:1, 1:2], zr[:])
        osc = const.tile([1, 1], f32, tag="osc")
        nc.vector.scalar_tensor_tensor(
            osc[:], fin[0:1, 2:3], 1.0 / M, p1t[:], op0=MULT, op1=ADD)
        nc.sync.dma_start(out[:], osc[:])

    nc.compile()
    return nc


def _split(v):
    h = v.astype(bf16)
    l = (v - h.astype(np.float32)).astype(bf16)
    return h, l


def _query_aug(P):
    """P [n,3] f32 -> [15, n] bf16 (lhsT / stationary side)."""
    rows = []
    for dd in range(3):
        h, l = _split(P[:, dd])
        rows += [h, h, l, l]
    one = np.ones(P.shape[0], dtype=bf16)
    rows += [one, one, one]
    return np.stack(rows, 0)


def _ref_aug(Q):
    """Q [m,3] f32 -> [15, m] bf16 (rhs / moving side, carries -2x and sq)."""
    rows = []
    eff = np.zeros(Q.shape, np.float64)
    for dd in range(3):
        h, l = _split(Q[:, dd])
        h2 = (-2.0 * h.astype(np.float32)).astype(bf16)
        l2 = (-2.0 * l.astype(np.float32)).astype(bf16)
        rows += [h2, l2, h2, l2]
        eff[:, dd] = h.astype(np.float64) + l.astype(np.float64)
    sq = (eff ** 2).sum(-1).astype(np.float32)
    s0 = sq.astype(bf16)
    r = sq - s0.astype(np.float32)
    s1 = r.astype(bf16)
    s2 = (r - s1.astype(np.float32)).astype(bf16)
    rows += [s0, s1, s2]
    return np.stack(rows, 0)


def _sq_eff(P):
    eff = np.zeros(P.shape, np.float64)
    for dd in range(3):
        h, l = _split(P[:, dd])
        eff[:, dd] = h.astype(np.float64) + l.astype(np.float64)
    return (eff ** 2).sum(-1).astype(np.float32)


def kernel(points1, points2, weights):
    global _compiled, _last_results
    from concourse.bass_utils import run_bass_kernel_spmd

    p1 = np.ascontiguousarray(np.asarray(points1, dtype=np.float32))
    p2 = np.ascontiguousarray(np.asarray(points2, dtype=np.float32))
    w = np.ascontiguousarray(np.asarray(weights, dtype=np.float32))

    if _compiled is None:
        _compiled = _build()

    in_maps = []
    for c in range(8):
        b, q = divmod(c, 4)
        p1b, p2b, wb = p1[b], p2[b], w[b]
        n0, m0 = q * NSH, q * MSH
        sq1 = _sq_eff(p1b[n0:n0 + NSH])
        sq2 = _sq_eff(p2b[m0:m0 + MSH])
        in_maps.append({
            "q1": np.ascontiguousarray(_query_aug(p1b[n0:n0 + NSH])),
            "r2": np.ascontiguousarray(_ref_aug(p2b)),
            "q2": np.ascontiguousarray(_query_aug(p2b[m0:m0 + MSH])),
            "r1": np.ascontiguousarray(_ref_aug(p1b)),
            "s1a": np.ascontiguousarray(sq1.reshape(NRT1, 128).T),
            "s2a": np.ascontiguousarray(sq2.reshape(NRT2, 128).T),
            "wmat": np.ascontiguousarray(wb.reshape(128, 128)),
            "wsh": np.ascontiguousarray(wb[n0:n0 + NSH].reshape(NSH // 128, 128)),
        })

    trace = os.environ.get("CHAMFER_TRACE", "0") == "1"
    res = run_bass_kernel_spmd(_compiled, in_maps, core_ids=list(range(8)),
                               trace=trace)
    _last_results = res
    parts = [float(res.results[c]["out"][0, 0]) for c in range(8)]
    return np.asarray(np.float32(sum(parts) / B))


# revision 9
# speedup vs baseline: 1.0084x; 1.0084x over previous
"""Chamfer loss (adapted) on 8 TRN2 NeuronCores via Bass/Tile.

Problem: B=2, N=16384, M=8192, D=3
  w = softmax(weights, axis=1)
  dist[b,n,m] = ||p1[b,n] - p2[b,m]||^2  (via sq1 + sq2 - 2*cross)
  loss = mean_b( sum_n w*min_m dist + mean_m min_n dist )

Sharding: core c -> batch b = c//4, quarter q = c%4.
  mm1: rows = p1[b, q*4096:(q+1)*4096] vs all M points2 -> min over free dim
  mm2: rows = p2[b, q*2048:(q+1)*2048] vs all N points1 -> min over free dim
No collectives: each core emits a partial scalar; host sums 8 partials / B.

Numerics: distances need ~1e-4 abs accuracy but are computed via terms of
magnitude ~O(10) (catastrophic cancellation), so the cross term cannot use
raw bf16 matmul. Each coordinate x is split hi/lo (x ~= xh + xl, both bf16);
the K=15 augmented contraction computes
  R[n,m] = -2*sum_d x_d*y_d + ||y||^2   exactly over the bf16-split points,
accumulated in fp32 PSUM (bf16*bf16 products are exact in fp32). min1[n] =
sq1[n] + min_m R[n,m]. TensorE streams 1 column/cycle regardless of K, so
K=15 costs the same as K=5 but keeps full precision.

Reduction (the bottleneck): TensorE produces 512 fp32/instr but
tensor_reduce runs at 1 elem/cycle/lane on VectorE. Per 2048-col block
either:
  path A: DVE reduce_min straight from PSUM (fp32)           ~2258ns
  path B: ScalarE Identity(+sq1 bias) PSUM->SBUF bf16 convert ~1850ns
          + DVE bf16 2x-mode pairwise-min tournament          ~1120ns
Mixing A and B balances ScalarE vs VectorE so both engines stay busy.
"""

import os
import numpy as np
import ml_dtypes

bf16 = ml_dtypes.bfloat16

B, N, M, D = 2, 16384, 8192, 3
NSH, MSH = N // 4, M // 4          # 4096 query rows (mm1), 2048 (mm2) per core
K = 15                             # augmented contraction depth
BLK = 2048                         # free-dim columns per PSUM block
NRT1, NBLK1 = NSH // 128, M // BLK     # 32 row-tiles x 4 blocks  (mm1)
NRT2, NBLK2 = MSH // 128, N // BLK     # 16 row-tiles x 8 blocks  (mm2)

# Fraction of row-tiles whose block 0 uses path A (DVE-direct reduce):
# measured rates: pathA costs DVE 2258ns/blk vs pathB 1670ns DVE + 1917ns ACT,
# so A only pays off for ~10% of blocks (keeps ScalarE and VectorE balanced).
PATH_A_EVERY_MM1 = 3   # rt % 3 == 0 -> block 0 is path A
PATH_A_EVERY_MM2 = 2

_compiled = None
_last_results = None


def _build():
    from contextlib import ExitStack
    import concourse.mybir as mybir
    import concourse.tile as tile
    from concourse import bacc
    from concourse.masks import make_identity

    f32, bf = mybir.dt.float32, mybir.dt.bfloat16
    X = mybir.AxisListType.X
    MIN, ADD, MULT = mybir.AluOpType.min, mybir.AluOpType.add, mybir.AluOpType.mult
    IDENT, EXP = mybir.ActivationFunctionType.Identity, mybir.ActivationFunctionType.Exp

    nc = bacc.Bacc("TRN2", target_bir_lowering=False, debug=False)

    q1 = nc.dram_tensor("q1", (K, NSH), bf, kind="ExternalInput").ap()
    r2 = nc.dram_tensor("r2", (K, M), bf, kind="ExternalInput").ap()
    q2 = nc.dram_tensor("q2", (K, MSH), bf, kind="ExternalInput").ap()
    r1 = nc.dram_tensor("r1", (K, N), bf, kind="ExternalInput").ap()
    s1a = nc.dram_tensor("s1a", (128, NRT1), f32, kind="ExternalInput").ap()
    s2a = nc.dram_tensor("s2a", (128, NRT2), f32, kind="ExternalInput").ap()
    wmat = nc.dram_tensor("wmat", (128, 128), f32, kind="ExternalInput").ap()
    wsh = nc.dram_tensor("wsh", (NSH // 128, 128), f32, kind="ExternalInput").ap()
    out = nc.dram_tensor("out", (1, 1), f32, kind="ExternalOutput").ap()

    with tile.TileContext(nc) as tc, ExitStack() as ctx:
        const = ctx.enter_context(tc.tile_pool(name="const", bufs=1))
        psum = ctx.enter_context(tc.tile_pool(name="psum", bufs=2, space="PSUM"))
        conv = ctx.enter_context(tc.tile_pool(name="conv", bufs=3))
        trn = ctx.enter_context(tc.tile_pool(name="trn", bufs=2))
        coll = ctx.enter_context(tc.tile_pool(name="coll", bufs=2))

        q1t = const.tile([K, NSH], bf, tag="q1t")
        nc.sync.dma_start(q1t[:], q1[:])
        r2t = const.tile([K, M], bf, tag="r2t")
        nc.sync.dma_start(r2t[:], r2[:])
        q2t = const.tile([K, MSH], bf, tag="q2t")
        nc.sync.dma_start(q2t[:], q2[:])
        r1t = const.tile([K, N], bf, tag="r1t")
        nc.sync.dma_start(r1t[:], r1[:])
        s1t = const.tile([128, NRT1], f32, tag="s1t")
        nc.sync.dma_start(s1t[:], s1a[:])
        s2t = const.tile([128, NRT2], f32, tag="s2t")
        nc.sync.dma_start(s2t[:], s2a[:])
        wmt = const.tile([128, 128], f32, tag="wmt")
        nc.sync.dma_start(wmt[:], wmat[:])
        wst = const.tile([NSH // 128, 128], f32, tag="wst")
        nc.sync.dma_start(wst[:], wsh[:])

        min1 = const.tile([128, NRT1], f32, tag="min1")
        min2 = const.tile([128, NRT2], f32, tag="min2")

        # PE clock warm-up: ~16 dense matmuls (>3.4us of PE activity) push the
        # HAM clock gate from 1.2GHz to 2.4GHz before the real stream starts.
        # Results are never read; WAW deps keep them back-to-back on the PE.
        wm = psum.tile([128, BLK], f32, tag="blk")
        for i in range(16):
            nc.tensor.matmul(wm[:, (i % 4) * 512:(i % 4 + 1) * 512],
                             q1t[:, 0:128], r2t[:, 0:512],
                             start=True, stop=True)

        def do_matrix(qt, rhs, rt_cnt, blk_cnt, bias, out_min, a_every):
            for rt in range(rt_cnt):
                lhsT = qt[:, rt * 128:(rt + 1) * 128]
                bias_col = bias[:, rt:rt + 1]
                a_blocks = (0,) if rt % a_every == 0 else ()
                n_b = blk_cnt - len(a_blocks)
                collB = coll.tile([128, 8 * 256], bf, tag="collB")
                collA = coll.tile([128, 8], f32, tag="collA")
                ia = ib = 0
                for j in range(blk_cnt):
                    ps = psum.tile([128, BLK], f32, tag="blk")
                    for k in range(4):
                        nc.tensor.matmul(
                            ps[:, k * 512:(k + 1) * 512], lhsT,
                            rhs[:, (j * 4 + k) * 512:(j * 4 + k + 1) * 512],
                            start=True, stop=True)
                    if j in a_blocks:
                        nc.vector.tensor_reduce(
                            collA[:, ia:ia + 1], ps[:], axis=X, op=MIN)
                        ia += 1
                    else:
                        cv = conv.tile([128, BLK], bf, tag="cv")
                        nc.scalar.activation(cv[:], ps[:], IDENT,
                                             bias=bias_col, scale=1.0)
                        t1 = trn.tile([128, 1024], bf, tag="t1")
                        nc.vector.tensor_tensor(
                            t1[:], cv[:, 0:1024], cv[:, 1024:2048], op=MIN)
                        t2 = trn.tile([128, 512], bf, tag="t2")
                        nc.vector.tensor_tensor(
                            t2[:], t1[:, 0:512], t1[:, 512:1024], op=MIN)
                        nc.vector.tensor_tensor(
                            collB[:, ib * 256:(ib + 1) * 256],
                            t2[:, 0:256], t2[:, 256:512], op=MIN)
                        ib += 1
                rB = coll.tile([128, 1], f32, tag="rB")
                nc.vector.tensor_reduce(
                    rB[:], collB[:, 0:n_b * 256], axis=X, op=MIN)
                if ia == 0:
                    nc.vector.tensor_copy(out_min[:, rt:rt + 1], rB[:])
                else:
                    rA = collA[:, 0:1]
                    # out = min(rA + bias, rB)  (pathB mins already biased)
                    nc.vector.scalar_tensor_tensor(
                        out_min[:, rt:rt + 1], rA, bias_col, rB[:],
                        op0=ADD, op1=MIN)

        do_matrix(q1t, r2t, NRT1, NBLK1, s1t, min1, PATH_A_EVERY_MM1)
        do_matrix(q2t, r1t, NRT2, NBLK2, s2t, min2, PATH_A_EVERY_MM2)

        # ---- finalization: softmax weights + weighted sums -> scalar ----
        ident = const.tile([128, 128], f32, tag="ident")
        make_identity(nc, ident[:])
        ewm = const.tile([128, 128], f32, tag="ewm")
        nc.scalar.activation(ewm[:], wmt[:], EXP)
        zcol = const.tile([128, 1], f32, tag="zcol")
        nc.vector.tensor_reduce(zcol[:], ewm[:], axis=X, op=ADD)
        ones = const.tile([128, 1], f32, tag="ones")
        nc.gpsimd.memset(ones[:], 1.0)

        wse = const.tile([NSH // 128, 128], f32, tag="wse")
        nc.scalar.activation(wse[:], wst[:], EXP)

        pz = psum.tile([128, BLK], f32, tag="blk")
        # Z = sum_n exp(w[n])    (cross-partition sum via PE)
        nc.tensor.matmul(pz[0:1, 0:1], zcol[:], ones[:], start=True, stop=True)
        # exp(w_shard) transposed into min1's [p, rt] layout
        nc.tensor.transpose(pz[0:128, 512:512 + NSH // 128], wse[:],
                            ident[0:NSH // 128, 0:NSH // 128])
        ewsh = const.tile([128, NRT1], f32, tag="ewsh")
        nc.scalar.copy(ewsh[:], pz[0:128, 512:512 + NRT1])

        tmp = const.tile([128, NRT1], f32, tag="tmp")
        t1v = const.tile([128, 1], f32, tag="t1v")
        nc.vector.scalar_tensor_tensor(
            tmp[:], ewsh[:], 1.0, min1[:], op0=MULT, op1=MULT,
            accum_out=t1v[:])
        t2v = const.tile([128, 1], f32, tag="t2v")
        nc.vector.tensor_reduce(t2v[:], min2[:], axis=X, op=ADD)
        nc.tensor.matmul(pz[0:1, 1024:1025], t1v[:], ones[:],
                         start=True, stop=True)
        nc.tensor.matmul(pz[0:1, 1536:1537], t2v[:], ones[:],
                         start=True, stop=True)

        fin = const.tile([1, 4], f32, tag="fin")
        nc.scalar.copy(fin[0:1, 0:1], pz[0:1, 0:1])
        nc.scalar.copy(fin[0:1, 1:2], pz[0:1, 1024:1025])
        nc.scalar.copy(fin[0:1, 2:3], pz[0:1, 1536:1537])
        zr = const.tile([1, 1], f32, tag="zr")
        nc.vector.reciprocal(zr[:], fin[0:1, 0:1])
        p1t = const.tile([1, 1], f32, tag="p1t")
        nc.vector.tensor_mul(p1t[:], fin[0:1, 1:2], zr[:])
        osc = const.tile([1, 1], f32, tag="osc")
        nc.vector.scalar_tensor_tensor(
            osc[:], fin[0:1, 2:3], 1.0 / M, p1t[:], op0=MULT, op1=ADD)
        nc.sync.dma_start(out[:], osc[:])

    nc.compile()
    return nc


def _split(v):
    h = v.astype(bf16)
    l = (v - h.astype(np.float32)).astype(bf16)
    return h, l


def _query_aug(P):
    """P [n,3] f32 -> [15, n] bf16 (lhsT / stationary side)."""
    rows = []
    for dd in range(3):
        h, l = _split(P[:, dd])
        rows += [h, h, l, l]
    one = np.ones(P.shape[0], dtype=bf16)
    rows += [one, one, one]
    return np.stack(rows, 0)


def _ref_aug(Q):
    """Q [m,3] f32 -> [15, m] bf16 (rhs / moving side, carries -2x and sq)."""
    rows = []
    eff = np.zeros(Q.shape, np.float64)
    for dd in range(3):
        h, l = _split(Q[:, dd])
        h2 = (-2.0 * h.astype(np.float32)).astype(bf16)
        l2 = (-2.0 * l.astype(np.float32)).astype(bf16)
        rows += [h2, l2, h2, l2]
        eff[:, dd] = h.astype(np.float64) + l.astype(np.float64)
    sq = (eff ** 2).sum(-1).astype(np.float32)
    s0 = sq.astype(bf16)
    r = sq - s0.astype(np.float32)
    s1 = r.astype(bf16)
    s2 = (r - s1.astype(np.float32)).astype(bf16)
    rows += [s0, s1, s2]
    return np.stack(rows, 0)


def _sq_eff(P):
    eff = np.zeros(P.shape, np.float64)
    for dd in range(3):
        h, l = _split(P[:, dd])
        eff[:, dd] = h.astype(np.float64) + l.astype(np.float64)
    return (eff ** 2).sum(-1).astype(np.float32)


def kernel(points1, points2, weights):
    global _compiled, _last_results
    from concourse.bass_utils import run_bass_kernel_spmd

    p1 = np.ascontiguousarray(np.asarray(points1, dtype=np.float32))
    p2 = np.ascontiguousarray(np.asarray(points2, dtype=np.float32))
    w = np.ascontiguousarray(np.asarray(weights, dtype=np.float32))

    if _compiled is None:
        _compiled = _build()

    in_maps = []
    for c in range(8):
        b, q = divmod(c, 4)
        p1b, p2b, wb = p1[b], p2[b], w[b]
        n0, m0 = q * NSH, q * MSH
        sq1 = _sq_eff(p1b[n0:n0 + NSH])
        sq2 = _sq_eff(p2b[m0:m0 + MSH])
        in_maps.append({
            "q1": np.ascontiguousarray(_query_aug(p1b[n0:n0 + NSH])),
            "r2": np.ascontiguousarray(_ref_aug(p2b)),
            "q2": np.ascontiguousarray(_query_aug(p2b[m0:m0 + MSH])),
            "r1": np.ascontiguousarray(_ref_aug(p1b)),
            "s1a": np.ascontiguousarray(sq1.reshape(NRT1, 128).T),
            "s2a": np.ascontiguousarray(sq2.reshape(NRT2, 128).T),
            "wmat": np.ascontiguousarray(wb.reshape(128, 128)),
            "wsh": np.ascontiguousarray(wb[n0:n0 + NSH].reshape(NSH // 128, 128)),
        })

    trace = os.environ.get("CHAMFER_TRACE", "0") == "1"
    res = run_bass_kernel_spmd(_compiled, in_maps, core_ids=list(range(8)),
                               trace=trace)
    _last_results = res
    parts = [float(res.results[c]["out"][0, 0]) for c in range(8)]
    return np.asarray(np.float32(sum(parts) / B))


# revision 10
# speedup vs baseline: 1.3531x; 1.3418x over previous
"""Chamfer loss (adapted) on 8 TRN2 NeuronCores via Bass/Tile.

Problem: B=2, N=16384, M=8192, D=3
  w = softmax(weights, axis=1)
  dist[b,n,m] = ||p1[b,n] - p2[b,m]||^2  (via sq1 + sq2 - 2*cross)
  loss = mean_b( sum_n w*min_m dist + mean_m min_n dist )

Sharding: core c -> batch b = c//4, quarter q = c%4. Each core computes the
distance matrix ONCE for rows n in its quarter (4096) x all M=8192 columns:
  min1 (row mins)    -> per-core, no communication
  min2 (column mins) -> partial mins over the core's rows, then
                        AllReduce-min across the batch's 4 cores
This halves TensorE work vs computing both (N,M) and (M,N) matrices, and the
bf16 dist tiles produced for min1 are reused for min2.

Numerics: distances need ~1e-4 abs accuracy but the terms are O(10)
(catastrophic cancellation), so the cross term cannot use raw bf16 matmul.
Each coordinate x is split hi/lo (x ~= xh + xl, both bf16); the K=15
augmented contraction computes R[n,m] = -2*sum_d x_d*y_d + ||y||^2 exactly
over the bf16-split points, accumulated in fp32 PSUM (bf16*bf16 products are
exact in fp32). TensorE streams 1 column/cycle regardless of K, so K=15
costs the same as K=5 but keeps full precision. ScalarE converts PSUM->SBUF
bf16 while adding the per-row sq1 bias, so the bf16 rounding happens on the
small biased distance (validated: final rel err ~1e-5).

Engine budget per core (measured rates): ScalarE converts 128 blocks x
1.92us = 246us; VectorE tournament row-mins + running column-mins ~360us;
TensorE 512 matmuls ~220us at the cold 1.2GHz clock (hidden).
"""

import os
import numpy as np
import ml_dtypes

bf16 = ml_dtypes.bfloat16

B, N, M, D = 2, 16384, 8192, 3
NSH = N // 4                       # 4096 query rows per core
K = 15                             # augmented contraction depth
BLK = 2048                         # free-dim columns per PSUM block
NRT, NBLK = NSH // 128, M // BLK   # 32 row-tiles x 4 blocks

# min2 running-min ownership: block handled by GpSimd when
# (rt*NBLK+j) % GPSIMD_MIN2_MOD == 0 (0 disables GpSimd).
GPSIMD_MIN2_MOD = int(os.environ.get("CHAMFER_GPSIMD_MOD", "0"))

_compiled = None
_last_results = None


def _build():
    from contextlib import ExitStack
    import concourse.mybir as mybir
    import concourse.tile as tile
    from concourse import bacc
    from concourse.masks import make_identity

    f32, bf = mybir.dt.float32, mybir.dt.bfloat16
    X = mybir.AxisListType.X
    MIN, ADD, MULT = mybir.AluOpType.min, mybir.AluOpType.add, mybir.AluOpType.mult
    IDENT, EXP = mybir.ActivationFunctionType.Identity, mybir.ActivationFunctionType.Exp

    nc = bacc.Bacc("TRN2", target_bir_lowering=False, debug=False, num_devices=8)

    q1 = nc.dram_tensor("q1", (K, NSH), bf, kind="ExternalInput").ap()
    r2 = nc.dram_tensor("r2", (K, M), bf, kind="ExternalInput").ap()
    s1a = nc.dram_tensor("s1a", (128, NRT), f32, kind="ExternalInput").ap()
    wmat = nc.dram_tensor("wmat", (128, 128), f32, kind="ExternalInput").ap()
    wsh = nc.dram_tensor("wsh", (NSH // 128, 128), f32, kind="ExternalInput").ap()
    out = nc.dram_tensor("out", (1, 1), f32, kind="ExternalOutput").ap()

    with tile.TileContext(nc) as tc, ExitStack() as ctx:
        const = ctx.enter_context(tc.tile_pool(name="const", bufs=1))
        psum = ctx.enter_context(tc.tile_pool(name="psum", bufs=2, space="PSUM"))
        conv = ctx.enter_context(tc.tile_pool(name="conv", bufs=3))
        trn = ctx.enter_context(tc.tile_pool(name="trn", bufs=2))
        coll = ctx.enter_context(tc.tile_pool(name="coll", bufs=2))
        dram = ctx.enter_context(tc.tile_pool(name="dram", bufs=1, space="DRAM"))

        q1t = const.tile([K, NSH], bf, tag="q1t")
        nc.sync.dma_start(q1t[:], q1[:])
        r2t = const.tile([K, M], bf, tag="r2t")
        nc.sync.dma_start(r2t[:], r2[:])
        s1t = const.tile([128, NRT], f32, tag="s1t")
        nc.sync.dma_start(s1t[:], s1a[:])
        wmt = const.tile([128, 128], f32, tag="wmt")
        nc.sync.dma_start(wmt[:], wmat[:])
        wst = const.tile([NSH // 128, 128], f32, tag="wst")
        nc.sync.dma_start(wst[:], wsh[:])

        min1 = const.tile([128, NRT], f32, tag="min1")
        acc = const.tile([128, M], bf, tag="acc")    # running column mins

        # PE clock warm-up: dense matmuls (>3.4us) push the HAM clock gate
        # toward 2.4GHz before the real stream starts. Never read back.
        wm = psum.tile([128, BLK], f32, tag="blk")
        for i in range(16):
            nc.tensor.matmul(wm[:, (i % 4) * 512:(i % 4 + 1) * 512],
                             q1t[:, 0:128], r2t[:, 0:512],
                             start=True, stop=True)

        gp_ctr = 0
        for rt in range(NRT):
            lhsT = q1t[:, rt * 128:(rt + 1) * 128]
            bias_col = s1t[:, rt:rt + 1]
            collB = coll.tile([128, NBLK * 256], bf, tag="collB")
            for j in range(NBLK):
                ps = psum.tile([128, BLK], f32, tag="blk")
                for k in range(4):
                    nc.tensor.matmul(
                        ps[:, k * 512:(k + 1) * 512], lhsT,
                        r2t[:, (j * 4 + k) * 512:(j * 4 + k + 1) * 512],
                        start=True, stop=True)
                # convert + bias: cv = bf16(R + sq1[row]) = bf16(dist)
                cv = conv.tile([128, BLK], bf, tag="cv")
                nc.scalar.activation(cv[:], ps[:], IDENT,
                                     bias=bias_col, scale=1.0)
                # row-min tournament (min1)
                t1 = trn.tile([128, 1024], bf, tag="t1")
                nc.vector.tensor_tensor(
                    t1[:], cv[:, 0:1024], cv[:, 1024:2048], op=MIN)
                t2 = trn.tile([128, 512], bf, tag="t2")
                nc.vector.tensor_tensor(
                    t2[:], t1[:, 0:512], t1[:, 512:1024], op=MIN)
                nc.vector.tensor_tensor(
                    collB[:, j * 256:(j + 1) * 256],
                    t2[:, 0:256], t2[:, 256:512], op=MIN)
                # running column-min (min2)
                acc_sl = acc[:, j * BLK:(j + 1) * BLK]
                if rt == 0:
                    nc.vector.tensor_copy(acc_sl, cv[:])
                else:
                    gp_ctr += 1
                    eng = (nc.gpsimd if (GPSIMD_MIN2_MOD and
                                         gp_ctr % GPSIMD_MIN2_MOD == 0)
                           else nc.vector)
                    eng.tensor_tensor(acc_sl, acc_sl, cv[:], op=MIN)
            nc.vector.tensor_reduce(
                min1[:, rt:rt + 1], collB[:], axis=X, op=MIN)

        # ---- min2 tail: fold partitions via PE transpose + reduce ----
        identb = const.tile([128, 128], bf, tag="identb")
        make_identity(nc, identb[:])
        min2t = const.tile([128, 64], f32, tag="min2t")
        for g in range(8):                     # 8 groups x 8 col-blocks
            pt = psum.tile([128, BLK], f32, tag="blk")
            ptb = pt[:].bitcast(bf)            # [128, 4096] bf16 view
            for kk in range(8):
                cb = g * 8 + kk
                nc.tensor.transpose(ptb[:, kk * 128:(kk + 1) * 128],
                                    acc[:, cb * 128:(cb + 1) * 128],
                                    identb[:])
            nc.vector.tensor_reduce(
                min2t[:, g * 8:(g + 1) * 8],
                ptb[:, 0:1024].rearrange("p (b f) -> p b f", f=128),
                axis=X, op=MIN)

        # ---- AllReduce-min of min2 partials across the batch's 4 cores ----
        bin_ = dram.tile([128, 64], f32, tag="bin")
        bout = dram.tile([128, 64], f32, tag="bout")
        nc.sync.dma_start(bin_[:], min2t[:])
        nc.gpsimd.collective_compute(
            "AllReduce", MIN,
            replica_groups=[[0, 1, 2, 3], [4, 5, 6, 7]],
            ins=[bin_[:].opt()], outs=[bout[:].opt()])
        min2r = const.tile([128, 64], f32, tag="min2r")
        nc.sync.dma_start(min2r[:], bout[:])

        # ---- softmax weights + weighted sums -> partial scalar ----
        ewm = const.tile([128, 128], f32, tag="ewm")
        nc.scalar.activation(ewm[:], wmt[:], EXP)
        zcol = const.tile([128, 1], f32, tag="zcol")
        nc.vector.tensor_reduce(zcol[:], ewm[:], axis=X, op=ADD)
        ones = const.tile([128, 1], f32, tag="ones")
        nc.gpsimd.memset(ones[:], 1.0)
        wse = const.tile([NSH // 128, 128], f32, tag="wse")
        nc.scalar.activation(wse[:], wst[:], EXP)

        pz = psum.tile([128, BLK], f32, tag="blk")
        # Z = sum_n exp(w[n])  (cross-partition sum via PE)
        nc.tensor.matmul(pz[0:1, 0:1], zcol[:], ones[:], start=True, stop=True)
        # exp(w_shard) transposed into min1's [p, rt] layout (fp32 transpose
        # needs an fp32 identity; reuse PE with bf16 identity is invalid)
        identf = const.tile([32, 32], f32, tag="identf")
        make_identity(nc, identf[:])
        nc.tensor.transpose(pz[0:128, 512:512 + NSH // 128], wse[:],
                            identf[:])
        ewsh = const.tile([128, NRT], f32, tag="ewsh")
        nc.scalar.copy(ewsh[:], pz[0:128, 512:512 + NRT])

        tmp = const.tile([128, NRT], f32, tag="tmp")
        t1v = const.tile([128, 1], f32, tag="t1v")
        nc.vector.scalar_tensor_tensor(
            tmp[:], ewsh[:], 1.0, min1[:], op0=MULT, op1=MULT,
            accum_out=t1v[:])
        t2v = const.tile([128, 1], f32, tag="t2v")
        nc.vector.tensor_reduce(t2v[:], min2r[:], axis=X, op=ADD)
        nc.tensor.matmul(pz[0:1, 1024:1025], t1v[:], ones[:],
                         start=True, stop=True)
        nc.tensor.matmul(pz[0:1, 1536:1537], t2v[:], ones[:],
                         start=True, stop=True)

        fin = const.tile([1, 4], f32, tag="fin")
        nc.scalar.copy(fin[0:1, 0:1], pz[0:1, 0:1])
        nc.scalar.copy(fin[0:1, 1:2], pz[0:1, 1024:1025])
        nc.scalar.copy(fin[0:1, 2:3], pz[0:1, 1536:1537])
        zr = const.tile([1, 1], f32, tag="zr")
        nc.vector.reciprocal(zr[:], fin[0:1, 0:1])
        p1t = const.tile([1, 1], f32, tag="p1t")
        nc.vector.tensor_mul(p1t[:], fin[0:1, 1:2], zr[:])
        osc = const.tile([1, 1], f32, tag="osc")
        # term2 counted once per core; each batch has 4 cores -> /(4M)
        nc.vector.scalar_tensor_tensor(
            osc[:], fin[0:1, 2:3], 1.0 / (4.0 * M), p1t[:],
            op0=MULT, op1=ADD)
        nc.sync.dma_start(out[:], osc[:])

    nc.compile()
    return nc


def _split(v):
    h = v.astype(bf16)
    l = (v - h.astype(np.float32)).astype(bf16)
    return h, l


def _query_aug(P):
    """P [n,3] f32 -> [15, n] bf16 (lhsT / stationary side)."""
    rows = []
    for dd in range(3):
        h, l = _split(P[:, dd])
        rows += [h, h, l, l]
    one = np.ones(P.shape[0], dtype=bf16)
    rows += [one, one, one]
    return np.stack(rows, 0)


def _ref_aug(Q):
    """Q [m,3] f32 -> [15, m] bf16 (rhs / moving side, carries -2y and sq)."""
    rows = []
    eff = np.zeros(Q.shape, np.float64)
    for dd in range(3):
        h, l = _split(Q[:, dd])
        h2 = (-2.0 * h.astype(np.float32)).astype(bf16)
        l2 = (-2.0 * l.astype(np.float32)).astype(bf16)
        rows += [h2, l2, h2, l2]
        eff[:, dd] = h.astype(np.float64) + l.astype(np.float64)
    sq = (eff ** 2).sum(-1).astype(np.float32)
    s0 = sq.astype(bf16)
    r = sq - s0.astype(np.float32)
    s1 = r.astype(bf16)
    s2 = (r - s1.astype(np.float32)).astype(bf16)
    rows += [s0, s1, s2]
    return np.stack(rows, 0)


def _sq_eff(P):
    eff = np.zeros(P.shape, np.float64)
    for dd in range(3):
        h, l = _split(P[:, dd])
        eff[:, dd] = h.astype(np.float64) + l.astype(np.float64)
    return (eff ** 2).sum(-1).astype(np.float32)


def kernel(points1, points2, weights):
    global _compiled, _last_results
    from concourse.bass_utils import run_bass_kernel_spmd

    p1 = np.ascontiguousarray(np.asarray(points1, dtype=np.float32))
    p2 = np.ascontiguousarray(np.asarray(points2, dtype=np.float32))
    w = np.ascontiguousarray(np.asarray(weights, dtype=np.float32))

    if _compiled is None:
        _compiled = _build()

    in_maps = []
    for c in range(8):
        b, q = divmod(c, 4)
        p1b, p2b, wb = p1[b], p2[b], w[b]
        n0 = q * NSH
        sq1 = _sq_eff(p1b[n0:n0 + NSH])
        in_maps.append({
            "q1": np.ascontiguousarray(_query_aug(p1b[n0:n0 + NSH])),
            "r2": np.ascontiguousarray(_ref_aug(p2b)),
            "s1a": np.ascontiguousarray(sq1.reshape(NRT, 128).T),
            "wmat": np.ascontiguousarray(wb.reshape(128, 128)),
            "wsh": np.ascontiguousarray(wb[n0:n0 + NSH].reshape(NSH // 128, 128)),
        })

    trace = os.environ.get("CHAMFER_TRACE", "0") == "1"
    res = run_bass_kernel_spmd(_compiled, in_maps, core_ids=list(range(8)),
                               trace=trace)
    _last_results = res
    parts = [float(res.results[c]["out"][0, 0]) for c in range(8)]
    return np.asarray(np.float32(sum(parts) / B))


# revision 13
# speedup vs baseline: 1.4850x; 1.0974x over previous
"""Chamfer loss (adapted) on 8 TRN2 NeuronCores via Bass/Tile.

Problem: B=2, N=16384, M=8192, D=3
  w = softmax(weights, axis=1)
  dist[b,n,m] = ||p1[b,n] - p2[b,m]||^2  (via sq1 + sq2 - 2*cross)
  loss = mean_b( sum_n w*min_m dist + mean_m min_n dist )

Sharding: core c -> batch b = c//4, quarter q = c%4. Each core computes the
distance matrix ONCE for rows n in its quarter (4096) x all M=8192 columns:
  min1 (row mins)    -> per-core, no communication
  min2 (column mins) -> partial mins over the core's rows, then
                        AllReduce-min across the batch's 4 cores
This halves TensorE work vs computing both (N,M) and (M,N) matrices, and the
bf16 dist tiles produced for min1 are reused for min2.

Numerics: distances need ~1e-4 abs accuracy but the terms are O(10)
(catastrophic cancellation), so the cross term cannot use raw bf16 matmul.
Each coordinate x is split hi/lo (x ~= xh + xl, both bf16); the K=15
augmented contraction computes R[n,m] = -2*sum_d x_d*y_d + ||y||^2 exactly
over the bf16-split points, accumulated in fp32 PSUM (bf16*bf16 products are
exact in fp32). TensorE streams 1 column/cycle regardless of K, so K=15
costs the same as K=5 but keeps full precision. ScalarE converts PSUM->SBUF
bf16 while adding the per-row sq1 bias, so the bf16 rounding happens on the
small biased distance (validated: final rel err ~1e-5).

Engine budget per core (measured rates): ScalarE converts 128 blocks x
1.92us = 246us; VectorE tournament row-mins + running column-mins ~360us;
TensorE 512 matmuls ~220us at the cold 1.2GHz clock (hidden).
"""

import os
import numpy as np
import ml_dtypes

bf16 = ml_dtypes.bfloat16

B, N, M, D = 2, 16384, 8192, 3
NSH = N // 4                       # 4096 query rows per core
K = 15                             # augmented contraction depth
BLK = 2048                         # free-dim columns per PSUM block
NRT, NBLK = NSH // 128, M // BLK   # 32 row-tiles x 4 blocks

# Columns of the min2 running-min handled by GpSimd (rest on VectorE).
# NOTE: walrus rejects InstTensorTensor on the Pool engine for TRN2
# ("Instruction engine check failed (Pool)"), so this stays 0.
GP_COLS = int(os.environ.get("CHAMFER_GP_COLS", "0"))

_compiled = None
_last_results = None


def _build():
    from contextlib import ExitStack
    import concourse.mybir as mybir
    import concourse.tile as tile
    from concourse import bacc
    from concourse.masks import make_identity

    f32, bf = mybir.dt.float32, mybir.dt.bfloat16
    X = mybir.AxisListType.X
    MIN, ADD, MULT = mybir.AluOpType.min, mybir.AluOpType.add, mybir.AluOpType.mult
    IDENT, EXP = mybir.ActivationFunctionType.Identity, mybir.ActivationFunctionType.Exp

    nc = bacc.Bacc("TRN2", target_bir_lowering=False, debug=False, num_devices=8)

    q1 = nc.dram_tensor("q1", (K, NSH), bf, kind="ExternalInput").ap()
    r2 = nc.dram_tensor("r2", (K, M), bf, kind="ExternalInput").ap()
    s1a = nc.dram_tensor("s1a", (128, NRT), f32, kind="ExternalInput").ap()
    wmat = nc.dram_tensor("wmat", (128, 128), f32, kind="ExternalInput").ap()
    wsh = nc.dram_tensor("wsh", (NSH // 128, 128), f32, kind="ExternalInput").ap()
    out = nc.dram_tensor("out", (1, 1), f32, kind="ExternalOutput").ap()

    with tile.TileContext(nc) as tc, ExitStack() as ctx:
        const = ctx.enter_context(tc.tile_pool(name="const", bufs=1))
        psum = ctx.enter_context(tc.tile_pool(name="psum", bufs=2, space="PSUM"))
        conv = ctx.enter_context(tc.tile_pool(name="conv", bufs=3))
        trn = ctx.enter_context(tc.tile_pool(name="trn", bufs=2))
        coll = ctx.enter_context(tc.tile_pool(name="coll", bufs=2))
        dram = ctx.enter_context(tc.tile_pool(name="dram", bufs=1, space="DRAM"))

        q1t = const.tile([K, NSH], bf, tag="q1t")
        nc.sync.dma_start(q1t[:], q1[:])
        r2t = const.tile([K, M], bf, tag="r2t")
        nc.sync.dma_start(r2t[:], r2[:])
        s1t = const.tile([128, NRT], f32, tag="s1t")
        nc.sync.dma_start(s1t[:], s1a[:])
        wmt = const.tile([128, 128], f32, tag="wmt")
        nc.sync.dma_start(wmt[:], wmat[:])
        wst = const.tile([NSH // 128, 128], f32, tag="wst")
        nc.sync.dma_start(wst[:], wsh[:])

        min1 = const.tile([128, NRT], f32, tag="min1")
        acc = const.tile([128, M], bf, tag="acc")    # running column mins

        # PE clock warm-up: dense matmuls (>3.4us) push the HAM clock gate
        # toward 2.4GHz before the real stream starts. Never read back.
        wm = psum.tile([128, BLK], f32, tag="blk")
        for i in range(16):
            nc.tensor.matmul(wm[:, (i % 4) * 512:(i % 4 + 1) * 512],
                             q1t[:, 0:128], r2t[:, 0:512],
                             start=True, stop=True)

        for rt in range(NRT):
            lhsT = q1t[:, rt * 128:(rt + 1) * 128]
            bias_col = s1t[:, rt:rt + 1]
            cvrow = conv.tile([128, M], bf, tag="cvrow")
            for j in range(NBLK):
                ps = psum.tile([128, BLK], f32, tag="blk")
                for k in range(4):
                    nc.tensor.matmul(
                        ps[:, k * 512:(k + 1) * 512], lhsT,
                        r2t[:, (j * 4 + k) * 512:(j * 4 + k + 1) * 512],
                        start=True, stop=True)
                # convert + bias: cv = bf16(R + sq1[row]) = bf16(dist)
                nc.scalar.activation(cvrow[:, j * BLK:(j + 1) * BLK], ps[:],
                                     IDENT, bias=bias_col, scale=1.0)
            # running column-min (min2), split VectorE / GpSimd by columns
            if rt == 0:
                nc.vector.tensor_copy(acc[:], cvrow[:])
            else:
                dv_cols = M - GP_COLS
                if dv_cols:
                    nc.vector.tensor_tensor(
                        acc[:, 0:dv_cols], acc[:, 0:dv_cols],
                        cvrow[:, 0:dv_cols], op=MIN)
                if GP_COLS:
                    nc.gpsimd.tensor_tensor(
                        acc[:, dv_cols:M], acc[:, dv_cols:M],
                        cvrow[:, dv_cols:M], op=MIN)
            # row-min tournament (min1) over the whole row-tile
            t1 = trn.tile([128, M // 2], bf, tag="t1")
            nc.vector.tensor_tensor(
                t1[:], cvrow[:, 0:M // 2], cvrow[:, M // 2:M], op=MIN)
            t2 = trn.tile([128, M // 4], bf, tag="t2")
            nc.vector.tensor_tensor(
                t2[:], t1[:, 0:M // 4], t1[:, M // 4:M // 2], op=MIN)
            t3 = trn.tile([128, M // 8], bf, tag="t3")
            nc.vector.tensor_tensor(
                t3[:], t2[:, 0:M // 8], t2[:, M // 8:M // 4], op=MIN)
            nc.vector.tensor_reduce(
                min1[:, rt:rt + 1], t3[:], axis=X, op=MIN)

        # ---- min2 tail: fold partitions via PE transpose + reduce ----
        identb = const.tile([128, 128], bf, tag="identb")
        make_identity(nc, identb[:])
        min2t = const.tile([128, 64], f32, tag="min2t")
        for g in range(8):                     # 8 groups x 8 col-blocks
            pt = psum.tile([128, BLK], f32, tag="blk")
            ptb = pt[:].bitcast(bf)            # [128, 4096] bf16 view
            for kk in range(8):
                cb = g * 8 + kk
                nc.tensor.transpose(ptb[:, kk * 128:(kk + 1) * 128],
                                    acc[:, cb * 128:(cb + 1) * 128],
                                    identb[:])
            nc.vector.tensor_reduce(
                min2t[:, g * 8:(g + 1) * 8],
                ptb[:, 0:1024].rearrange("p (b f) -> p b f", f=128),
                axis=X, op=MIN)

        # ---- AllReduce-min of min2 partials across the batch's 4 cores ----
        bin_ = dram.tile([128, 64], f32, tag="bin")
        bout = dram.tile([128, 64], f32, tag="bout")
        nc.sync.dma_start(bin_[:], min2t[:])
        nc.gpsimd.collective_compute(
            "AllReduce", MIN,
            replica_groups=[[0, 1, 2, 3], [4, 5, 6, 7]],
            ins=[bin_[:].opt()], outs=[bout[:].opt()])
        min2r = const.tile([128, 64], f32, tag="min2r")
        nc.sync.dma_start(min2r[:], bout[:])

        # ---- softmax weights + weighted sums -> partial scalar ----
        ewm = const.tile([128, 128], f32, tag="ewm")
        nc.scalar.activation(ewm[:], wmt[:], EXP)
        zcol = const.tile([128, 1], f32, tag="zcol")
        nc.vector.tensor_reduce(zcol[:], ewm[:], axis=X, op=ADD)
        ones = const.tile([128, 1], f32, tag="ones")
        nc.gpsimd.memset(ones[:], 1.0)
        wse = const.tile([NSH // 128, 128], f32, tag="wse")
        nc.scalar.activation(wse[:], wst[:], EXP)

        pz = psum.tile([128, BLK], f32, tag="blk")
        # Z = sum_n exp(w[n])  (cross-partition sum via PE)
        nc.tensor.matmul(pz[0:1, 0:1], zcol[:], ones[:], start=True, stop=True)
        # exp(w_shard) transposed into min1's [p, rt] layout (fp32 transpose
        # needs an fp32 identity; reuse PE with bf16 identity is invalid)
        identf = const.tile([32, 32], f32, tag="identf")
        make_identity(nc, identf[:])
        nc.tensor.transpose(pz[0:128, 512:512 + NSH // 128], wse[:],
                            identf[:])
        ewsh = const.tile([128, NRT], f32, tag="ewsh")
        nc.scalar.copy(ewsh[:], pz[0:128, 512:512 + NRT])

        tmp = const.tile([128, NRT], f32, tag="tmp")
        t1v = const.tile([128, 1], f32, tag="t1v")
        nc.vector.scalar_tensor_tensor(
            tmp[:], ewsh[:], 1.0, min1[:], op0=MULT, op1=MULT,
            accum_out=t1v[:])
        t2v = const.tile([128, 1], f32, tag="t2v")
        nc.vector.tensor_reduce(t2v[:], min2r[:], axis=X, op=ADD)
        nc.tensor.matmul(pz[0:1, 1024:1025], t1v[:], ones[:],
                         start=True, stop=True)
        nc.tensor.matmul(pz[0:1, 1536:1537], t2v[:], ones[:],
                         start=True, stop=True)

        fin = const.tile([1, 4], f32, tag="fin")
        nc.scalar.copy(fin[0:1, 0:1], pz[0:1, 0:1])
        nc.scalar.copy(fin[0:1, 1:2], pz[0:1, 1024:1025])
        nc.scalar.copy(fin[0:1, 2:3], pz[0:1, 1536:1537])
        zr = const.tile([1, 1], f32, tag="zr")
        nc.vector.reciprocal(zr[:], fin[0:1, 0:1])
        p1t = const.tile([1, 1], f32, tag="p1t")
        nc.vector.tensor_mul(p1t[:], fin[0:1, 1:2], zr[:])
        osc = const.tile([1, 1], f32, tag="osc")
        # term2 counted once per core; each batch has 4 cores -> /(4M)
        nc.vector.scalar_tensor_tensor(
            osc[:], fin[0:1, 2:3], 1.0 / (4.0 * M), p1t[:],
            op0=MULT, op1=ADD)
        nc.sync.dma_start(out[:], osc[:])

    nc.compile()
    return nc


def _split(v):
    h = v.astype(bf16)
    l = (v - h.astype(np.float32)).astype(bf16)
    return h, l


def _query_aug(P):
    """P [n,3] f32 -> [15, n] bf16 (lhsT / stationary side)."""
    rows = []
    for dd in range(3):
        h, l = _split(P[:, dd])
        rows += [h, h, l, l]
    one = np.ones(P.shape[0], dtype=bf16)
    rows += [one, one, one]
    return np.stack(rows, 0)


def _ref_aug(Q):
    """Q [m,3] f32 -> [15, m] bf16 (rhs / moving side, carries -2y and sq)."""
    rows = []
    eff = np.zeros(Q.shape, np.float64)
    for dd in range(3):
        h, l = _split(Q[:, dd])
        h2 = (-2.0 * h.astype(np.float32)).astype(bf16)
        l2 = (-2.0 * l.astype(np.float32)).astype(bf16)
        rows += [h2, l2, h2, l2]
        eff[:, dd] = h.astype(np.float64) + l.astype(np.float64)
    sq = (eff ** 2).sum(-1).astype(np.float32)
    s0 = sq.astype(bf16)
    r = sq - s0.astype(np.float32)
    s1 = r.astype(bf16)
    s2 = (r - s1.astype(np.float32)).astype(bf16)
    rows += [s0, s1, s2]
    return np.stack(rows, 0)


def _sq_eff(P):
    eff = np.zeros(P.shape, np.float64)
    for dd in range(3):
        h, l = _split(P[:, dd])
        eff[:, dd] = h.astype(np.float64) + l.astype(np.float64)
    return (eff ** 2).sum(-1).astype(np.float32)


def kernel(points1, points2, weights):
    global _compiled, _last_results
    from concourse.bass_utils import run_bass_kernel_spmd

    p1 = np.ascontiguousarray(np.asarray(points1, dtype=np.float32))
    p2 = np.ascontiguousarray(np.asarray(points2, dtype=np.float32))
    w = np.ascontiguousarray(np.asarray(weights, dtype=np.float32))

    if _compiled is None:
        _compiled = _build()

    in_maps = []
    for c in range(8):
        b, q = divmod(c, 4)
        p1b, p2b, wb = p1[b], p2[b], w[b]
        n0 = q * NSH
        sq1 = _sq_eff(p1b[n0:n0 + NSH])
        in_maps.append({
            "q1": np.ascontiguousarray(_query_aug(p1b[n0:n0 + NSH])),
            "r2": np.ascontiguousarray(_ref_aug(p2b)),
            "s1a": np.ascontiguousarray(sq1.reshape(NRT, 128).T),
            "wmat": np.ascontiguousarray(wb.reshape(128, 128)),
            "wsh": np.ascontiguousarray(wb[n0:n0 + NSH].reshape(NSH // 128, 128)),
        })

    trace = os.environ.get("CHAMFER_TRACE", "0") == "1"
    res = run_bass_kernel_spmd(_compiled, in_maps, core_ids=list(range(8)),
                               trace=trace)
    _last_results = res
    parts = [float(res.results[c]["out"][0, 0]) for c in range(8)]
    return np.asarray(np.float32(sum(parts) / B))
